# revision 17
# baseline (speedup 1.0000x reference)
"""Trainium2 Bass kernel for nn_DiagonalTraining (ragged per-anti-diagonal linear).

Math (reference): for each batch image x[b] (SxS) and each anti-diagonal
i (elements x[b, r, i-r], r=0..i), apply a per-diagonal linear layer:
  out[b,i,q] = sum_{r<=i} x[b,r,i-r] * W[i,q,r] + bias[i,q]   (q <= i)
and scatter back: y[b,q,i-q] = out[b,i,q]; positions with r+c >= S keep x.

Distribution: diagonal i -> core i%8, slot j=i//8 (64 slots per core,
balanced by construction). Host packs, per (core, slot), an augmented
matrix whose rows are the contraction axis r:
  [ D^T | V ]  with D^T[r,b]=x[b,r,i-r], V[r,q]=W[i,q,r]  (r,q < ni=i+1)
zero-padded to a core-independent size NJ=8*(j+1) (>= ni for every
core) so the SPMD program is identical on all cores. The per-diagonal
bias is added on the host while scattering results back (elementwise,
~0.05% of the FLOPs; the whole einsum runs on device).

Device ("window streaming"): each slot is split into row-chunks of up
to 128 rows; chunk columns ([*, 32+NJ] blocks) are packed first-fit
into window tiles. Full 128-row chunks go to [128, <=WF] windows; the
per-slot partial (<128-row) chunk goes to a row-class window ([32|64|
96|120, *]) keyed by (row class, quartile j//16), so the DMA ships
almost no dead padding rows and each partial window is consumed
within ~one psum group of where it loads. Windows are loaded by big
SWDGE DMAs (one descriptor per partition row, WF*dtype bytes each) —
spread evenly over all 16 SDMA engines, streaming at near-HBM rate,
fully decoupled from compute. Matmuls read chunks at static (window,
column) offsets, accumulating psum[32, NJ] per slot inside a
bank-packed 4-slot group psum tile; one DVE copy per group stages
results, and all group stores run at the end of the SWDGE queue.

Only the live (lower-triangular) part of W is shipped/read, in bf16
(~13 MB/core vs 512 MB full f32 W) — the kernel is HBM-bound on ~those
bytes. bf16 matmul streams 1 column/cycle; rel-err stays ~1e-3 vs the
2e-2 gate (products accumulate in f32 PSUM).
"""

import sys

for _p in ("/opt/trn_rl_repo", "/opt/pypackages"):
    if _p not in sys.path:
        sys.path.append(_p)

import numpy as np

import concourse.bass as bass  # noqa: F401
import concourse.tile as tile
from concourse import bacc, mybir
from concourse.bass_utils import run_bass_kernel_spmd

B = 32          # batch
S = 512         # seq len / number of diagonals
N_CORES = 8
N_SLOTS = S // N_CORES  # 64 slots per core
DCOL = B        # width of the D^T block (batch on matmul M axis)
GROUP = 4       # slots per psum group
N_GROUPS = N_SLOTS // GROUP
WF = 3072       # window free size (elems per partition)

KCFG = {
    "compute": "bf16",  # "f32" | "f32r" | "bf16"
    "out": "bf16",      # "f32" | "bf16"
    "win_bufs": 12,
    "psum_bufs": 2,
}

# ---- static layout ----------------------------------------------------
# processing order: largest slot first
_ORDER = list(range(N_SLOTS - 1, -1, -1))
_GROUPS = [_ORDER[g * GROUP : (g + 1) * GROUP] for g in range(N_GROUPS)]


# chunk placement. Full 128-row chunks go to 128-row windows (small
# first windows so the first matmuls start early, then WF-wide).
# Partial chunks go to a row-class window (rows = smallest of
# 32/64/96/120 that fits), so the window DMA ships at most 24 dead
# rows per chunk instead of up to 120. Class windows are as wide as
# full ones (6 KiB descriptors — small descriptors measurably drop
# per-engine DMA throughput on HW) and span many psum groups, so they
# are pinned in a dedicated pool for the whole run instead of rotating
# through the streaming pool. Window ids are assigned in first-use
# order (the walk below is emission order), so DMA order tracks
# consumption order.
_SLOT_CHUNKS = {j: [] for j in range(N_SLOTS)}  # j -> [(win, cbase, rows, row_start)]
_WIN_ROWS = []  # partition rows per window
_WIN_W = []     # running (exact) width per window
_WIN_CAP = []   # column capacity per window
_cls_win = {}   # allocation key -> open window id
_n_full_wins = 0


def _open_win(rows, cap):
    _WIN_ROWS.append(rows)
    _WIN_W.append(0)
    _WIN_CAP.append(cap)
    return len(_WIN_ROWS) - 1


def _alloc_block(key, rows, wd):
    global _n_full_wins
    w = _cls_win.get(key)
    if w is None or _WIN_W[w] + wd > _WIN_CAP[w]:
        if key == "full":
            cap = (1024, 2048)[_n_full_wins] if _n_full_wins < 2 else WF
            _n_full_wins += 1
        else:
            cap = WF
        w = _open_win(rows, cap)
        _cls_win[key] = w
    cb = _WIN_W[w]
    _WIN_W[w] += wd
    return w, cb


for _j in _ORDER:
    _NJ = 8 * (_j + 1)
    _wd = DCOL + _NJ
    _nfull = _NJ // 128
    for _c in range(_nfull):
        _w, _cb = _alloc_block("full", 128, _wd)
        _SLOT_CHUNKS[_j].append((_w, _cb, 128, 128 * _c))
    _pr = _NJ % 128
    if _pr == 0:
        continue
    _rc = next(rc for rc in (32, 64, 96, 120) if _pr <= rc)
    _w, _cb = _alloc_block(_rc, _rc, _wd)
    _SLOT_CHUNKS[_j].append((_w, _cb, _pr, 128 * _nfull))

N_WINS = len(_WIN_ROWS)
# round widths to 32 elems so every window's descriptor size and DRAM
# base stay 64-byte aligned
for _w in range(N_WINS):
    _WIN_W[_w] = -(-_WIN_W[_w] // 32) * 32
N_PIN_WINS = sum(1 for _r in _WIN_ROWS if _r < 128)
_WIN_OFF = []
_boff = 0
for _w in range(N_WINS):
    _WIN_OFF.append(_boff)
    _boff += _WIN_ROWS[_w] * _WIN_W[_w]
BLOB_ELEMS = _boff

# psum group column layout (bank-aligned, no matmul straddles a bank)
_BANK = 512
_GROUP_COLS = []
_GROUP_W = []
for _slots in _GROUPS:
    _col = 0
    _cols = []
    for _j in _slots:
        _NJ = 8 * (_j + 1)
        if _col // _BANK != (_col + _NJ - 1) // _BANK:
            _col = ((_col + _BANK - 1) // _BANK) * _BANK
        _cols.append((_j, _col))
        _col += _NJ
    _GROUP_COLS.append(_cols)
    _GROUP_W.append(_col)

_GOUT_OFF = []
_SLOT_OUT = {}
_goff = 0
for _g in range(N_GROUPS):
    _GOUT_OFF.append(_goff)
    for _j, _col in _GROUP_COLS[_g]:
        _SLOT_OUT[_j] = (_g, _col)
    _goff += B * _GROUP_W[_g]
OUT_ELEMS = _goff

_compiled = {}


def _build_program():
    key = (KCFG["compute"], KCFG["out"], KCFG["win_bufs"], KCFG["psum_bufs"])
    if key in _compiled:
        return _compiled[key]

    from contextlib import ExitStack

    nc = bacc.Bacc("TRN2", target_bir_lowering=False, debug=False)
    f32 = mybir.dt.float32
    mm_dt = {
        "f32": f32,
        "f32r": mybir.dt.float32r,
        "bf16": mybir.dt.bfloat16,
    }[KCFG["compute"]]
    out_dt = {"f32": f32, "bf16": mybir.dt.bfloat16}[KCFG["out"]]
    blob = nc.dram_tensor("blob", [BLOB_ELEMS], mm_dt, kind="ExternalInput").ap()
    outb = nc.dram_tensor("outblob", [OUT_ELEMS], out_dt, kind="ExternalOutput").ap()

    with tile.TileContext(nc) as tc, ExitStack() as ctx:
        win_pool = ctx.enter_context(
            tc.tile_pool(name="win", bufs=KCFG["win_bufs"])
        )
        pin_pool = ctx.enter_context(
            tc.tile_pool(name="pinwin", bufs=max(N_PIN_WINS, 1))
        )
        acc_pool = ctx.enter_context(tc.tile_pool(name="acc", bufs=1))
        psum_pool = ctx.enter_context(
            tc.tile_pool(name="psum", bufs=KCFG["psum_bufs"], space="PSUM")
        )

        # window tiles are loaded lazily in program order; keep handles
        win_tiles = [None] * N_WINS

        def ensure_win(w):
            if win_tiles[w] is None:
                wf = _WIN_W[w]
                rows = _WIN_ROWS[w]
                pool = win_pool if rows == 128 else pin_pool
                t = pool.tile([rows, wf], mm_dt)
                src = blob[_WIN_OFF[w] : _WIN_OFF[w] + rows * wf].rearrange(
                    "(p f) -> p f", p=rows, f=wf
                )
                nc.gpsimd.dma_start(t[:], src)
                win_tiles[w] = t
            return win_tiles[w]

        tot_w = OUT_ELEMS // B
        acc_t = acc_pool.tile([B, tot_w], out_dt)
        for g, slots in enumerate(_GROUPS):
            gw = _GROUP_W[g]
            gcol = _GOUT_OFF[g] // B
            psum_t = psum_pool.tile([B, gw], f32)
            for j, col in _GROUP_COLS[g]:
                NJ = 8 * (j + 1)
                wd = DCOL + NJ
                chs = _SLOT_CHUNKS[j]
                for c, (w, cb, rows, _rs) in enumerate(chs):
                    t = ensure_win(w)
                    nc.tensor.matmul(
                        psum_t[:, col : col + NJ],
                        t[0:rows, cb : cb + DCOL],
                        t[0:rows, cb + DCOL : cb + wd],
                        start=(c == 0),
                        stop=(c == len(chs) - 1),
                    )
            nc.vector.tensor_copy(acc_t[:, gcol : gcol + gw], psum_t[:])
        # staged stores: earlier group ranges flush while later compute
        # still runs. All after the loads on the Pool queue, so a store
        # wait only ever blocks later (even more dependent) stores.
        dstv = outb[:].rearrange("(p w) -> p w", p=B, w=tot_w)
        cuts = [0, _GOUT_OFF[8] // B, _GOUT_OFF[13] // B, tot_w]
        for a, bnd in zip(cuts, cuts[1:]):
            nc.gpsimd.dma_start(dstv[:, a:bnd], acc_t[:, a:bnd])

    nc.compile()
    _compiled[key] = nc
    return nc


def _np_dt():
    if KCFG["compute"] == "bf16":
        import ml_dtypes

        return ml_dtypes.bfloat16
    return np.float32


def _pack_core(k, x, W, bias):
    np_dt = _np_dt()
    blob = np.zeros(BLOB_ELEMS, np_dt)
    for j in range(N_SLOTS):
        i = N_CORES * j + k
        ni = i + 1
        NJ = 8 * (j + 1)
        wd = DCOL + NJ
        M = np.zeros((NJ, wd), np.float32)
        r = np.arange(ni)
        M[:ni, :DCOL] = x[:, r, i - r].T               # D^T[r, b]
        M[:ni, DCOL : DCOL + ni] = W[i, :ni, :ni].T    # V[r, q]
        for w, cb, rows, rs in _SLOT_CHUNKS[j]:
            rl = M[rs : rs + rows]                     # [rows, wd]
            wf = _WIN_W[w]
            wr = _WIN_ROWS[w]
            img = blob[_WIN_OFF[w] : _WIN_OFF[w] + wr * wf].reshape(wr, wf)
            img[0:rows, cb : cb + wd] = rl.astype(np_dt)
    return blob


def kernel(x, W, b):
    x = np.asarray(x, np.float32)
    W = np.asarray(W, np.float32)
    b = np.asarray(b, np.float32)

    nc = _build_program()
    in_maps = [{"blob": _pack_core(k, x, W, b)} for k in range(N_CORES)]
    res = run_bass_kernel_spmd(nc, in_maps, list(range(N_CORES)))

    y = x.copy()
    tot_w = OUT_ELEMS // B
    for k in range(N_CORES):
        ob = np.asarray(res.results[k]["outblob"], np.float32).reshape(B, tot_w)
        for j in range(N_SLOTS):
            i = N_CORES * j + k
            ni = i + 1
            g, col = _SLOT_OUT[j]
            gcol = _GOUT_OFF[g] // B + col
            q = np.arange(ni)
            y[:, q, i - q] = ob[:, gcol : gcol + ni] + b[i, :ni][None]
    return y


# revision 21
# speedup vs baseline: 1.0144x; 1.0144x over previous
"""Trainium2 Bass kernel for nn_DiagonalTraining (ragged per-anti-diagonal linear).

Math (reference): for each batch image x[b] (SxS) and each anti-diagonal
i (elements x[b, r, i-r], r=0..i), apply a per-diagonal linear layer:
  out[b,i,q] = sum_{r<=i} x[b,r,i-r] * W[i,q,r] + bias[i,q]   (q <= i)
and scatter back: y[b,q,i-q] = out[b,i,q]; positions with r+c >= S keep x.

Distribution: diagonal i -> core i%8, slot j=i//8 (64 slots per core,
balanced by construction). Host packs, per (core, slot), an augmented
matrix whose rows are the contraction axis r:
  [ D^T | V ]  with D^T[r,b]=x[b,r,i-r], V[r,q]=W[i,q,r]  (r,q < ni=i+1)
zero-padded to a core-independent size NJ=8*(j+1) (>= ni for every
core) so the SPMD program is identical on all cores. The per-diagonal
bias is added on the host while scattering results back (elementwise,
~0.05% of the FLOPs; the whole einsum runs on device).

Device ("window streaming"): each slot is split into row-chunks of up
to 128 rows; chunk columns ([*, 32+NJ] blocks) are packed first-fit
into window tiles. Full 128-row chunks go to [128, <=WF] windows; the
per-slot partial (<128-row) chunk goes to a row-class window ([32|64|
96|120, *]) keyed by (row class, quartile j//16), so the DMA ships
almost no dead padding rows and each partial window is consumed
within ~one psum group of where it loads. Windows are loaded by big
SWDGE DMAs (one descriptor per partition row, WF*dtype bytes each) —
spread evenly over all 16 SDMA engines, streaming at near-HBM rate,
fully decoupled from compute. Matmuls read chunks at static (window,
column) offsets, accumulating psum[32, NJ] per slot inside a
bank-packed 4-slot group psum tile; one DVE copy per group stages
results, and all group stores run at the end of the SWDGE queue.

Only the live (lower-triangular) part of W is shipped/read, in bf16
(~13 MB/core vs 512 MB full f32 W) — the kernel is HBM-bound on ~those
bytes. bf16 matmul streams 1 column/cycle; rel-err stays ~1e-3 vs the
2e-2 gate (products accumulate in f32 PSUM).
"""

import sys

for _p in ("/opt/trn_rl_repo", "/opt/pypackages"):
    if _p not in sys.path:
        sys.path.append(_p)

import numpy as np

import concourse.bass as bass  # noqa: F401
import concourse.tile as tile
from concourse import bacc, mybir
from concourse.bass_utils import run_bass_kernel_spmd

B = 32          # batch
S = 512         # seq len / number of diagonals
N_CORES = 8
N_SLOTS = S // N_CORES  # 64 slots per core
DCOL = B        # width of the D^T block (batch on matmul M axis)
GROUP = 4       # slots per psum group
N_GROUPS = N_SLOTS // GROUP
WF = 3072       # window free size (elems per partition)

KCFG = {
    "compute": "bf16",  # "f32" | "f32r" | "bf16"
    "out": "bf16",      # "f32" | "bf16"
    "win_bufs": 16,
    "psum_bufs": 8,
}

# ---- static layout ----------------------------------------------------
# processing order: largest slot first
_ORDER = list(range(N_SLOTS - 1, -1, -1))
_GROUPS = [_ORDER[g * GROUP : (g + 1) * GROUP] for g in range(N_GROUPS)]


# chunk placement. Full 128-row chunks go to 128-row windows (small
# first windows so the first matmuls start early, then WF-wide).
# Partial chunks go to a row-class window (rows = smallest of
# 32/64/96/120 that fits), so the window DMA ships at most 24 dead
# rows per chunk instead of up to 120. Class windows are as wide as
# full ones (6 KiB descriptors — small descriptors measurably drop
# per-engine DMA throughput on HW) and span many psum groups, so they
# are pinned in a dedicated pool for the whole run instead of rotating
# through the streaming pool. Window ids are assigned in first-use
# order (the walk below is emission order), so DMA order tracks
# consumption order.
_SLOT_CHUNKS = {j: [] for j in range(N_SLOTS)}  # j -> [(win, cbase, rows, row_start)]
_WIN_ROWS = []  # partition rows per window
_WIN_W = []     # running (exact) width per window
_WIN_CAP = []   # column capacity per window
_cls_win = {}   # allocation key -> open window id
_n_full_wins = 0


def _open_win(rows, cap):
    _WIN_ROWS.append(rows)
    _WIN_W.append(0)
    _WIN_CAP.append(cap)
    return len(_WIN_ROWS) - 1


def _alloc_block(key, rows, wd):
    global _n_full_wins
    w = _cls_win.get(key)
    if w is None or _WIN_W[w] + wd > _WIN_CAP[w]:
        if key == "full":
            cap = (1024, 2048)[_n_full_wins] if _n_full_wins < 2 else WF
            _n_full_wins += 1
        else:
            cap = WF
        w = _open_win(rows, cap)
        _cls_win[key] = w
    cb = _WIN_W[w]
    _WIN_W[w] += wd
    return w, cb


for _j in _ORDER:
    _NJ = 8 * (_j + 1)
    _wd = DCOL + _NJ
    _nfull = _NJ // 128
    for _c in range(_nfull):
        _w, _cb = _alloc_block("full", 128, _wd)
        _SLOT_CHUNKS[_j].append((_w, _cb, 128, 128 * _c))
    _pr = _NJ % 128
    if _pr == 0:
        continue
    _rc = next(rc for rc in (32, 64, 96, 120) if _pr <= rc)
    _w, _cb = _alloc_block(_rc, _rc, _wd)
    _SLOT_CHUNKS[_j].append((_w, _cb, _pr, 128 * _nfull))

N_WINS = len(_WIN_ROWS)
# round widths to 32 elems so every window's descriptor size and DRAM
# base stay 64-byte aligned
for _w in range(N_WINS):
    _WIN_W[_w] = -(-_WIN_W[_w] // 32) * 32
N_PIN_WINS = sum(1 for _r in _WIN_ROWS if _r < 128)
_WIN_OFF = []
_boff = 0
for _w in range(N_WINS):
    _WIN_OFF.append(_boff)
    _boff += _WIN_ROWS[_w] * _WIN_W[_w]
BLOB_ELEMS = _boff

# compact per-slot output columns, in processing order (one psum bank
# per slot, so no bank-straddle padding is needed)
_SLOT_OUT = {}
_ocol = 0
for _j in _ORDER:
    _SLOT_OUT[_j] = _ocol
    _ocol += 8 * (_j + 1)
TOT_W = _ocol
OUT_ELEMS = B * TOT_W

# staged stores: flush after these positions in _ORDER (the last cut is
# tiny, so the post-compute tail is short)
_STORE_CUTS = [16, 32, 48, N_SLOTS]

_compiled = {}


def _build_program():
    key = (KCFG["compute"], KCFG["out"], KCFG["win_bufs"], KCFG["psum_bufs"])
    if key in _compiled:
        return _compiled[key]

    from contextlib import ExitStack

    nc = bacc.Bacc("TRN2", target_bir_lowering=False, debug=False)
    f32 = mybir.dt.float32
    mm_dt = {
        "f32": f32,
        "f32r": mybir.dt.float32r,
        "bf16": mybir.dt.bfloat16,
    }[KCFG["compute"]]
    out_dt = {"f32": f32, "bf16": mybir.dt.bfloat16}[KCFG["out"]]
    blob = nc.dram_tensor("blob", [BLOB_ELEMS], mm_dt, kind="ExternalInput").ap()
    outb = nc.dram_tensor("outblob", [OUT_ELEMS], out_dt, kind="ExternalOutput").ap()

    with tile.TileContext(nc) as tc, ExitStack() as ctx:
        win_pool = ctx.enter_context(
            tc.tile_pool(name="win", bufs=KCFG["win_bufs"])
        )
        pin_pool = ctx.enter_context(
            tc.tile_pool(name="pinwin", bufs=max(N_PIN_WINS, 1))
        )
        acc_pool = ctx.enter_context(tc.tile_pool(name="acc", bufs=1))
        psum_pool = ctx.enter_context(
            tc.tile_pool(name="psum", bufs=KCFG["psum_bufs"], space="PSUM")
        )

        # window tiles are loaded lazily in program order; keep handles
        win_tiles = [None] * N_WINS

        def ensure_win(w):
            if win_tiles[w] is None:
                wf = _WIN_W[w]
                rows = _WIN_ROWS[w]
                pool = win_pool if rows == 128 else pin_pool
                t = pool.tile([rows, wf], mm_dt)
                src = blob[_WIN_OFF[w] : _WIN_OFF[w] + rows * wf].rearrange(
                    "(p f) -> p f", p=rows, f=wf
                )
                nc.gpsimd.dma_start(t[:], src)
                win_tiles[w] = t
            return win_tiles[w]

        # one psum tile (= one bank) per slot, 8 deep, so the PE almost
        # never waits on a psum->acc cast; casts alternate between the
        # Vector and Scalar engines so neither becomes the chain. Stores
        # go out mid-run on the Sync queue, so only the (tiny) last
        # store runs after the final compute.
        acc_t = acc_pool.tile([B, TOT_W], out_dt)
        dstv = outb[:].rearrange("(p w) -> p w", p=B, w=TOT_W)
        store_from = 0
        for idx, j in enumerate(_ORDER):
            NJ = 8 * (j + 1)
            wd = DCOL + NJ
            col = _SLOT_OUT[j]
            psum_t = psum_pool.tile([B, NJ], f32)
            chs = _SLOT_CHUNKS[j]
            for c, (w, cb, rows, _rs) in enumerate(chs):
                t = ensure_win(w)
                nc.tensor.matmul(
                    psum_t[:],
                    t[0:rows, cb : cb + DCOL],
                    t[0:rows, cb + DCOL : cb + wd],
                    start=(c == 0),
                    stop=(c == len(chs) - 1),
                )
            if idx % 2 == 0:
                nc.vector.tensor_copy(acc_t[:, col : col + NJ], psum_t[:])
            else:
                nc.scalar.copy(acc_t[:, col : col + NJ], psum_t[:])
            if idx + 1 in _STORE_CUTS:
                bnd = col + NJ
                nc.sync.dma_start(
                    dstv[:, store_from:bnd], acc_t[:, store_from:bnd]
                )
                store_from = bnd

    nc.compile()
    _compiled[key] = nc
    return nc


def _np_dt():
    if KCFG["compute"] == "bf16":
        import ml_dtypes

        return ml_dtypes.bfloat16
    return np.float32


def _pack_core(k, x, W, bias):
    np_dt = _np_dt()
    blob = np.zeros(BLOB_ELEMS, np_dt)
    for j in range(N_SLOTS):
        i = N_CORES * j + k
        ni = i + 1
        NJ = 8 * (j + 1)
        wd = DCOL + NJ
        M = np.zeros((NJ, wd), np.float32)
        r = np.arange(ni)
        M[:ni, :DCOL] = x[:, r, i - r].T               # D^T[r, b]
        M[:ni, DCOL : DCOL + ni] = W[i, :ni, :ni].T    # V[r, q]
        for w, cb, rows, rs in _SLOT_CHUNKS[j]:
            rl = M[rs : rs + rows]                     # [rows, wd]
            wf = _WIN_W[w]
            wr = _WIN_ROWS[w]
            img = blob[_WIN_OFF[w] : _WIN_OFF[w] + wr * wf].reshape(wr, wf)
            img[0:rows, cb : cb + wd] = rl.astype(np_dt)
    return blob


def kernel(x, W, b):
    x = np.asarray(x, np.float32)
    W = np.asarray(W, np.float32)
    b = np.asarray(b, np.float32)

    nc = _build_program()
    in_maps = [{"blob": _pack_core(k, x, W, b)} for k in range(N_CORES)]
    res = run_bass_kernel_spmd(nc, in_maps, list(range(N_CORES)))

    y = x.copy()
    for k in range(N_CORES):
        ob = np.asarray(res.results[k]["outblob"], np.float32).reshape(B, TOT_W)
        for j in range(N_SLOTS):
            i = N_CORES * j + k
            ni = i + 1
            col = _SLOT_OUT[j]
            q = np.arange(ni)
            y[:, q, i - q] = ob[:, col : col + ni] + b[i, :ni][None]
    return y


# revision 23
# speedup vs baseline: 1.0342x; 1.0196x over previous
"""Trainium2 Bass kernel for nn_DiagonalTraining (ragged per-anti-diagonal linear).

Math (reference): for each batch image x[b] (SxS) and each anti-diagonal
i (elements x[b, r, i-r], r=0..i), apply a per-diagonal linear layer:
  out[b,i,q] = sum_{r<=i} x[b,r,i-r] * W[i,q,r] + bias[i,q]   (q <= i)
and scatter back: y[b,q,i-q] = out[b,i,q]; positions with r+c >= S keep x.

Distribution: diagonal i -> core i%8, slot j=i//8 (64 slots per core,
balanced by construction). Host packs, per (core, slot), an augmented
matrix whose rows are the contraction axis r:
  [ D^T | V ]  with D^T[r,b]=x[b,r,i-r], V[r,q]=W[i,q,r]  (r,q < ni=i+1)
zero-padded to a core-independent size NJ=8*(j+1) (>= ni for every
core) so the SPMD program is identical on all cores. The per-diagonal
bias is added on the host while scattering results back (elementwise,
~0.05% of the FLOPs; the whole einsum runs on device).

Device ("window streaming"): each slot is split into row-chunks of up
to 128 rows; chunk columns ([*, 32+NJ] blocks) are packed first-fit
into window tiles. Full 128-row chunks go to [128, <=WF] windows; the
per-slot partial (<128-row) chunk goes to a row-class window ([32|64|
96|120, *]) keyed by (row class, quartile j//16), so the DMA ships
almost no dead padding rows and each partial window is consumed
within ~one psum group of where it loads. Windows are loaded by big
SWDGE DMAs (one descriptor per partition row, WF*dtype bytes each) —
spread evenly over all 16 SDMA engines, streaming at near-HBM rate,
fully decoupled from compute. Matmuls read chunks at static (window,
column) offsets, accumulating psum[32, NJ] per slot inside a
bank-packed 4-slot group psum tile; one DVE copy per group stages
results, and all group stores run at the end of the SWDGE queue.

Only the live (lower-triangular) part of W is shipped/read, in bf16
(~13 MB/core vs 512 MB full f32 W) — the kernel is HBM-bound on ~those
bytes. bf16 matmul streams 1 column/cycle; rel-err stays ~1e-3 vs the
2e-2 gate (products accumulate in f32 PSUM).
"""

import sys

for _p in ("/opt/trn_rl_repo", "/opt/pypackages"):
    if _p not in sys.path:
        sys.path.append(_p)

import numpy as np

import concourse.bass as bass  # noqa: F401
import concourse.tile as tile
from concourse import bacc, mybir
from concourse.bass_utils import run_bass_kernel_spmd

B = 32          # batch
S = 512         # seq len / number of diagonals
N_CORES = 8
N_SLOTS = S // N_CORES  # 64 slots per core
DCOL = B        # width of the D^T block (batch on matmul M axis)
GROUP = 4       # slots per psum group
N_GROUPS = N_SLOTS // GROUP
WF = 3072       # window free size (elems per partition)

KCFG = {
    "compute": "bf16",  # "f32" | "f32r" | "bf16"
    "out": "bf16",      # "f32" | "bf16"
    "win_bufs": 16,
    "psum_bufs": 8,
    # windows enqueued ahead of the first-consumed one: the PE starts
    # late with a deep SBUF backlog and then runs continuously, staying
    # at the ramped p-state instead of oscillating at DMA rate
    "delay": 7,
}

# ---- static layout ----------------------------------------------------
# processing order: largest slot first
_ORDER = list(range(N_SLOTS - 1, -1, -1))
_GROUPS = [_ORDER[g * GROUP : (g + 1) * GROUP] for g in range(N_GROUPS)]


# chunk placement. Full 128-row chunks go to 128-row windows (small
# first windows so the first matmuls start early, then WF-wide).
# Partial chunks go to a row-class window (rows = smallest of
# 32/64/96/120 that fits), so the window DMA ships at most 24 dead
# rows per chunk instead of up to 120. Class windows are as wide as
# full ones (6 KiB descriptors — small descriptors measurably drop
# per-engine DMA throughput on HW) and span many psum groups, so they
# are pinned in a dedicated pool for the whole run instead of rotating
# through the streaming pool. Window ids are assigned in first-use
# order (the walk below is emission order), so DMA order tracks
# consumption order.
_SLOT_CHUNKS = {j: [] for j in range(N_SLOTS)}  # j -> [(win, cbase, rows, row_start)]
_WIN_ROWS = []  # partition rows per window
_WIN_W = []     # running (exact) width per window
_WIN_CAP = []   # column capacity per window
_cls_win = {}   # allocation key -> open window id
_n_full_wins = 0


def _open_win(rows, cap):
    _WIN_ROWS.append(rows)
    _WIN_W.append(0)
    _WIN_CAP.append(cap)
    return len(_WIN_ROWS) - 1


def _alloc_block(key, rows, wd):
    global _n_full_wins
    w = _cls_win.get(key)
    if w is None or _WIN_W[w] + wd > _WIN_CAP[w]:
        if key == "full":
            cap = (1024, 2048)[_n_full_wins] if _n_full_wins < 2 else WF
            _n_full_wins += 1
        else:
            cap = WF
        w = _open_win(rows, cap)
        _cls_win[key] = w
    cb = _WIN_W[w]
    _WIN_W[w] += wd
    return w, cb


for _j in _ORDER:
    _NJ = 8 * (_j + 1)
    _wd = DCOL + _NJ
    _nfull = _NJ // 128
    for _c in range(_nfull):
        _w, _cb = _alloc_block("full", 128, _wd)
        _SLOT_CHUNKS[_j].append((_w, _cb, 128, 128 * _c))
    _pr = _NJ % 128
    if _pr == 0:
        continue
    _rc = next(rc for rc in (32, 64, 96, 120) if _pr <= rc)
    _w, _cb = _alloc_block(_rc, _rc, _wd)
    _SLOT_CHUNKS[_j].append((_w, _cb, _pr, 128 * _nfull))

N_WINS = len(_WIN_ROWS)
# round widths to 32 elems so every window's descriptor size and DRAM
# base stay 64-byte aligned
for _w in range(N_WINS):
    _WIN_W[_w] = -(-_WIN_W[_w] // 32) * 32
N_PIN_WINS = sum(1 for _r in _WIN_ROWS if _r < 128)
_WIN_OFF = []
_boff = 0
for _w in range(N_WINS):
    _WIN_OFF.append(_boff)
    _boff += _WIN_ROWS[_w] * _WIN_W[_w]
BLOB_ELEMS = _boff

# compact per-slot output columns, in processing order (one psum bank
# per slot, so no bank-straddle padding is needed)
_SLOT_OUT = {}
_ocol = 0
for _j in _ORDER:
    _SLOT_OUT[_j] = _ocol
    _ocol += 8 * (_j + 1)
TOT_W = _ocol
OUT_ELEMS = B * TOT_W

# staged stores: flush after these positions in _ORDER (the last cut is
# tiny, so the post-compute tail is short)
_STORE_CUTS = [16, 32, 48, N_SLOTS]

_compiled = {}


def _build_program():
    key = (KCFG["compute"], KCFG["out"], KCFG["win_bufs"], KCFG["psum_bufs"])
    if key in _compiled:
        return _compiled[key]

    from contextlib import ExitStack

    nc = bacc.Bacc("TRN2", target_bir_lowering=False, debug=False)
    f32 = mybir.dt.float32
    mm_dt = {
        "f32": f32,
        "f32r": mybir.dt.float32r,
        "bf16": mybir.dt.bfloat16,
    }[KCFG["compute"]]
    out_dt = {"f32": f32, "bf16": mybir.dt.bfloat16}[KCFG["out"]]
    blob = nc.dram_tensor("blob", [BLOB_ELEMS], mm_dt, kind="ExternalInput").ap()
    outb = nc.dram_tensor("outblob", [OUT_ELEMS], out_dt, kind="ExternalOutput").ap()

    with tile.TileContext(nc) as tc, ExitStack() as ctx:
        win_pool = ctx.enter_context(
            tc.tile_pool(name="win", bufs=KCFG["win_bufs"])
        )
        pin_pool = ctx.enter_context(
            tc.tile_pool(name="pinwin", bufs=max(N_PIN_WINS, 1))
        )
        acc_pool = ctx.enter_context(tc.tile_pool(name="acc", bufs=1))
        psum_pool = ctx.enter_context(
            tc.tile_pool(name="psum", bufs=KCFG["psum_bufs"], space="PSUM")
        )

        # window tiles are loaded lazily in program order; keep handles
        win_tiles = [None] * N_WINS

        def ensure_win(w):
            if win_tiles[w] is None:
                wf = _WIN_W[w]
                rows = _WIN_ROWS[w]
                pool = win_pool if rows == 128 else pin_pool
                t = pool.tile([rows, wf], mm_dt)
                src = blob[_WIN_OFF[w] : _WIN_OFF[w] + rows * wf].rearrange(
                    "(p f) -> p f", p=rows, f=wf
                )
                nc.gpsimd.dma_start(t[:], src)
                win_tiles[w] = t
            return win_tiles[w]

        # delayed start: enqueue loads for windows 1..delay BEFORE
        # window 0, so the first matmul (which needs window 0) waits
        # until `delay` windows are already buffered in SBUF
        for w in range(1, min(KCFG["delay"] + 1, N_WINS)):
            ensure_win(w)

        # one psum tile (= one bank) per slot, 8 deep, so the PE almost
        # never waits on a psum->acc cast; casts alternate between the
        # Vector and Scalar engines so neither becomes the chain. Stores
        # go out mid-run on the Sync queue, so only the (tiny) last
        # store runs after the final compute.
        acc_t = acc_pool.tile([B, TOT_W], out_dt)
        dstv = outb[:].rearrange("(p w) -> p w", p=B, w=TOT_W)
        store_from = 0
        for idx, j in enumerate(_ORDER):
            NJ = 8 * (j + 1)
            wd = DCOL + NJ
            col = _SLOT_OUT[j]
            psum_t = psum_pool.tile([B, NJ], f32)
            chs = _SLOT_CHUNKS[j]
            for c, (w, cb, rows, _rs) in enumerate(chs):
                t = ensure_win(w)
                nc.tensor.matmul(
                    psum_t[:],
                    t[0:rows, cb : cb + DCOL],
                    t[0:rows, cb + DCOL : cb + wd],
                    start=(c == 0),
                    stop=(c == len(chs) - 1),
                )
            if idx % 2 == 0:
                nc.vector.tensor_copy(acc_t[:, col : col + NJ], psum_t[:])
            else:
                nc.scalar.copy(acc_t[:, col : col + NJ], psum_t[:])
            if idx + 1 in _STORE_CUTS:
                bnd = col + NJ
                nc.sync.dma_start(
                    dstv[:, store_from:bnd], acc_t[:, store_from:bnd]
                )
                store_from = bnd

    nc.compile()
    _compiled[key] = nc
    return nc


def _np_dt():
    if KCFG["compute"] == "bf16":
        import ml_dtypes

        return ml_dtypes.bfloat16
    return np.float32


def _pack_core(k, x, W, bias):
    np_dt = _np_dt()
    blob = np.zeros(BLOB_ELEMS, np_dt)
    for j in range(N_SLOTS):
        i = N_CORES * j + k
        ni = i + 1
        NJ = 8 * (j + 1)
        wd = DCOL + NJ
        M = np.zeros((NJ, wd), np.float32)
        r = np.arange(ni)
        M[:ni, :DCOL] = x[:, r, i - r].T               # D^T[r, b]
        M[:ni, DCOL : DCOL + ni] = W[i, :ni, :ni].T    # V[r, q]
        for w, cb, rows, rs in _SLOT_CHUNKS[j]:
            rl = M[rs : rs + rows]                     # [rows, wd]
            wf = _WIN_W[w]
            wr = _WIN_ROWS[w]
            img = blob[_WIN_OFF[w] : _WIN_OFF[w] + wr * wf].reshape(wr, wf)
            img[0:rows, cb : cb + wd] = rl.astype(np_dt)
    return blob


def kernel(x, W, b):
    x = np.asarray(x, np.float32)
    W = np.asarray(W, np.float32)
    b = np.asarray(b, np.float32)

    nc = _build_program()
    in_maps = [{"blob": _pack_core(k, x, W, b)} for k in range(N_CORES)]
    res = run_bass_kernel_spmd(nc, in_maps, list(range(N_CORES)))

    y = x.copy()
    for k in range(N_CORES):
        ob = np.asarray(res.results[k]["outblob"], np.float32).reshape(B, TOT_W)
        for j in range(N_SLOTS):
            i = N_CORES * j + k
            ni = i + 1
            col = _SLOT_OUT[j]
            q = np.arange(ni)
            y[:, q, i - q] = ob[:, col : col + ni] + b[i, :ni][None]
    return y
